# revision 1
# baseline (speedup 1.0000x reference)
"""Dual-pixel depth-merge (forward splat) kernel for Trainium2, 8 NeuronCores.

Math: for integer pixel grid x, the reference computes pos = fl(x +- depth)
(f32-rounded), x0 = floor(pos), f = pos - x0. Define the per-view fractional
offsets
    v_l[i] = fl(i + depth[i]) - i   (exact f32 subtraction, in [0, 8])
    v_r[i] = i - fl(i - depth[i])   (exact f32 subtraction, in [0, 8])
Then each view's splat is a 9-tap shifted weighted sum with hat weights
    Wl_d = relu(1 - |v_l - d|),  Wr_d = relu(1 - |v_r - d|),  d = 0..8:
    count_l[j] = sum_d Wl_d[j-d]      acc_l[c,j] = sum_d (Wl_d*img_c)[j-d]
    count_r[j] = sum_d Wr_d[j+d]      acc_r[c,j] = sum_d (Wr_d*img_c)[j+d]
    left = acc_l / max(count_l, eps)  right = acc_r / max(count_r, eps)
This reproduces the reference's weights bit-for-bit (matching its f32
rounding of x+-depth), so count==0 happens exactly where the reference's
does — and there acc==0 too, making the eps-divide equal the reference's
where(count==0, 1, count).

Sharding: pure data parallel over h (the scatter is along w only) — core m
takes h rows [m*128, (m+1)*128) for all batches. No halo, no communication.
"""

import numpy as np

import concourse.bacc as bacc
import concourse.bass as bass
import concourse.mybir as mybir
import concourse.tile as tile
from concourse.bass_utils import run_bass_kernel_spmd

B, C, H, W = 4, 3, 1024, 1024
NCORES = 8
HS = H // NCORES  # 128 h-rows per core
NTAP = 9
F32 = mybir.dt.float32
EPS = 1e-20

_MAX = mybir.AluOpType.max
_ADD = mybir.AluOpType.add
_SUB = mybir.AluOpType.subtract
_RELU = mybir.ActivationFunctionType.Relu
_ABS = mybir.ActivationFunctionType.Abs
_IDENT = mybir.ActivationFunctionType.Identity


def _bcast_c(ap):
    """View a [HS, W] tile as [HS, C, W] by repeating along a step-0 dim."""
    a = ap.ap
    return bass.AP(tensor=ap.tensor, offset=ap.offset, ap=[list(a[0]), [0, C], list(a[1])])


CFG = {
    "io": 2, "w": 4, "v": 2, "t": 2, "p": 3, "acc": 2, "accn": 2,
    # Column split: DVE handles w in [0, spl), GPSIMD handles [spl, W), for
    # products/adds (spl_p) and count sums (spl_c).
    "spl_p": 688,
    "spl_a": 656,
    "spl_c": 688,
}


def build_program(skip: frozenset = frozenset(), cfg: dict | None = None) -> bass.Bass:
    """skip: debug knob — subset of {"counts", "prods", "weights"} to omit
    (produces wrong results; used only for critical-path bisection)."""
    cfg = {**CFG, **(cfg or {})}
    nc = bacc.Bacc()
    image = nc.dram_tensor("image", [B, C, HS, W], F32, kind="ExternalInput")
    depth = nc.dram_tensor("depth", [B, HS, W], F32, kind="ExternalInput")
    left = nc.dram_tensor("left", [B, C, HS, W], F32, kind="ExternalOutput")
    right = nc.dram_tensor("right", [B, C, HS, W], F32, kind="ExternalOutput")

    with tile.TileContext(nc) as tc:
        with (
            tc.tile_pool(name="consts", bufs=1) as c_pool,
            tc.tile_pool(name="io", bufs=cfg["io"]) as io_pool,
            tc.tile_pool(name="wts", bufs=cfg["w"]) as w_pool,
            tc.tile_pool(name="voff", bufs=cfg["v"]) as v_pool,
            tc.tile_pool(name="tmp", bufs=cfg["t"]) as t_pool,
            tc.tile_pool(name="prod", bufs=cfg["p"]) as p_pool,
            tc.tile_pool(name="accs", bufs=cfg["acc"]) as acc_pool,
            tc.tile_pool(name="accn", bufs=cfg["accn"]) as accn_pool,
        ):
            # Per-tap bias constants and the column-index (iota) row.
            negd = c_pool.tile([HS, NTAP], F32, tag="negd")
            for d in range(NTAP):
                nc.vector.memset(negd[:, d : d + 1], -float(d))
            epsb = c_pool.tile([HS, 1], F32, tag="epsb")
            nc.vector.memset(epsb[:], EPS)
            iota_i = t_pool.tile([HS, W], mybir.dt.int32, tag="t")
            nc.gpsimd.iota(iota_i[:], [[1, W]], channel_multiplier=0)
            iota = c_pool.tile([HS, W], F32, tag="iota")
            nc.vector.tensor_copy(iota[:], iota_i[:])

            for b in range(B):
                dep = io_pool.tile([HS, W], F32, tag="dep")
                nc.sync.dma_start(out=dep[:], in_=depth[b])
                img = io_pool.tile([HS, C, W], F32, tag="img")
                nc.sync.dma_start(out=img[:], in_=image[b].transpose([1, 0, 2]))

                # Exact per-view fractional offsets (reproduce reference's
                # f32 rounding of x +- depth; the second subtract is exact).
                vl = v_pool.tile([HS, W], F32, tag="vl")
                vr = v_pool.tile([HS, W], F32, tag="vr")
                s = t_pool.tile([HS, W], F32, tag="s")
                nc.vector.tensor_tensor(s[:], dep[:], iota[:], _ADD)
                nc.vector.tensor_tensor(vl[:], s[:], iota[:], _SUB)
                s2 = t_pool.tile([HS, W], F32, tag="s")
                nc.gpsimd.tensor_tensor(s2[:], iota[:], dep[:], _SUB)
                nc.gpsimd.tensor_tensor(vr[:], iota[:], s2[:], _SUB)

                # Interleave the two views tap-by-tap so DVE/GPSIMD/ACT all
                # stay fed. Work is column-split: DVE takes [0, spl),
                # GPSIMD [spl, W) of every product/add/count op.
                sp = cfg["spl_p"]
                sa = cfg["spl_a"]
                sc = cfg["spl_c"]
                views = (("l", vl), ("r", vr))
                cnt_l = accn_pool.tile([HS, W], F32, tag="cl")
                cnt_r = accn_pool.tile([HS, W], F32, tag="cr")
                acc_l = acc_pool.tile([HS, C, W], F32, tag="al")
                acc_r = acc_pool.tile([HS, C, W], F32, tag="ar")
                cnts = {"l": cnt_l, "r": cnt_r}
                accs = {"l": acc_l, "r": acc_r}
                for d in range(NTAP):
                    for view, v in views:
                        cnt, acc = cnts[view], accs[view]
                        # Tap weight W_d = relu(1 - |v - d|) on the scalar engine.
                        td = t_pool.tile([HS, W], F32, tag="t")
                        nc.scalar.activation(td[:], v[:], _ABS, bias=negd[:, d : d + 1], scale=1.0)
                        wd = w_pool.tile([HS, W], F32, tag="w")
                        nc.scalar.activation(wd[:], td[:], _RELU, bias=1.0, scale=-1.0)
                        if d == 0:
                            nc.scalar.copy(cnt[:], wd[:])
                            nc.vector.tensor_mul(acc[:, :, 0:sp], _bcast_c(wd[:, 0:sp]), img[:, :, 0:sp])
                            nc.gpsimd.tensor_mul(acc[:, :, sp:W], _bcast_c(wd[:, sp:W]), img[:, :, sp:W])
                            continue
                        if "counts" not in skip:
                            if view == "l":
                                nc.vector.tensor_tensor(cnt[:, d:sc], cnt[:, d:sc], wd[:, 0 : sc - d], _ADD)
                                nc.gpsimd.tensor_tensor(cnt[:, sc:W], cnt[:, sc:W], wd[:, sc - d : W - d], _ADD)
                            else:
                                nc.vector.tensor_tensor(cnt[:, 0:sc], cnt[:, 0:sc], wd[:, d : sc + d], _ADD)
                                nc.gpsimd.tensor_tensor(cnt[:, sc : W - d], cnt[:, sc : W - d], wd[:, sc + d : W], _ADD)
                        if "prods" not in skip:
                            pd = p_pool.tile([HS, C, W], F32, tag="p")
                            nc.vector.tensor_mul(pd[:, :, 0:sp], _bcast_c(wd[:, 0:sp]), img[:, :, 0:sp])
                            nc.gpsimd.tensor_mul(pd[:, :, sp:W], _bcast_c(wd[:, sp:W]), img[:, :, sp:W])
                            if view == "l":
                                nc.vector.tensor_add(acc[:, :, d:sa], acc[:, :, d:sa], pd[:, :, 0 : sa - d])
                                nc.gpsimd.tensor_add(acc[:, :, sa:W], acc[:, :, sa:W], pd[:, :, sa - d : W - d])
                            else:
                                nc.vector.tensor_add(acc[:, :, 0:sa], acc[:, :, 0:sa], pd[:, :, d : sa + d])
                                nc.gpsimd.tensor_add(acc[:, :, sa : W - d], acc[:, :, sa : W - d], pd[:, :, sa + d : W])

                # Normalize: out = acc * (1 / max(count, eps)).
                for view, _ in views:
                    cnt, acc = cnts[view], accs[view]
                    # count >= 0 and its smallest nonzero value is ~6e-8, so
                    # count + 1e-20 is bit-identical to max(count, 1e-20) —
                    # and an add-constant runs on the idle scalar engine.
                    rc = accn_pool.tile([HS, W], F32, tag=f"rc{view}")
                    nc.scalar.activation(cnt[:], cnt[:], _IDENT, bias=epsb[:], scale=1.0)
                    nc.vector.reciprocal_approx_fast(out=rc[:], in_=cnt[:])
                    nc.vector.tensor_mul(acc[:, :, 0:sa], acc[:, :, 0:sa], _bcast_c(rc[:, 0:sa]))
                    nc.gpsimd.tensor_mul(acc[:, :, sa:W], acc[:, :, sa:W], _bcast_c(rc[:, sa:W]))

                nc.sync.dma_start(out=left[b].transpose([1, 0, 2]), in_=accs["l"][:])
                nc.sync.dma_start(out=right[b].transpose([1, 0, 2]), in_=accs["r"][:])
    nc.compile()
    return nc


_NC_CACHE = None


def _get_program():
    global _NC_CACHE
    if _NC_CACHE is None:
        _NC_CACHE = build_program()
    return _NC_CACHE


def kernel(image: np.ndarray, depth: np.ndarray):
    image = np.ascontiguousarray(image, dtype=np.float32)
    depth = np.ascontiguousarray(depth, dtype=np.float32)
    assert image.shape == (B, C, H, W) and depth.shape == (B, H, W)

    nc = _get_program()
    in_maps = []
    for m in range(NCORES):
        sl = slice(m * HS, (m + 1) * HS)
        in_maps.append(
            {
                "image": np.ascontiguousarray(image[:, :, sl, :]),
                "depth": np.ascontiguousarray(depth[:, sl, :]),
            }
        )
    # The axon-tunneled devices occasionally come up in a transient
    # unrecoverable/desynced state (e.g. poisoned by a previous failed
    # process) and recover on the next attempt — retry once before giving up.
    try:
        res = run_bass_kernel_spmd(nc, in_maps, core_ids=list(range(NCORES)))
    except Exception:
        import time as _time

        _time.sleep(5.0)
        res = run_bass_kernel_spmd(nc, in_maps, core_ids=list(range(NCORES)))
    left = np.concatenate([r["left"] for r in res.results], axis=2)
    right = np.concatenate([r["right"] for r in res.results], axis=2)
    return left, right



# revision 10
# speedup vs baseline: 2.0057x; 2.0057x over previous
"""Dual-pixel depth-merge (forward splat) kernel for Trainium2, 8 NeuronCores.

Math: for integer pixel grid x, the reference computes pos = fl(x +- depth)
(f32-rounded), x0 = floor(pos), f = pos - x0. Define the per-view fractional
offsets
    v_l[i] = fl(i + depth[i]) - i   (exact f32 subtraction, in [0, 8])
    v_r[i] = i - fl(i - depth[i])   (exact f32 subtraction, in [0, 8])
Then each view's splat is a 9-tap shifted weighted sum with hat weights
    W_d = relu(1 - |v - d|), d = 0..8:
    count_l[j] = sum_d W_d[j-d]      acc_l[c,j] = sum_d (W_d*img_c)[j-d]
    count_r[j] = sum_d W_d[j+d]      acc_r[c,j] = sum_d (W_d*img_c)[j+d]
    out = acc / max(count, eps)

Implementation strategy (v2):
  - Work with NEGATED weights: -W_d = min(|v-d| - 1, 0). This needs no
    reverse-subtract and is exact in f32 (the zero set matches the reference
    bit-for-bit, which the count==0 -> divide-by-1 semantics require). Signs
    cancel at normalization: out = (-acc) * (-1/(count+eps)).
  - eps is folded into tap 0's weight: -W_0 = min(v-1, -eps) adds exactly
    -eps to every count column (and a negligible 1e-12 to acc).
  - Weights: ACT does t=|v-d| (abs, per-tap bias), DVE does the fused
    (t-1) min 0 tensor_scalar; both at [128, 2*1024] covering l+r at once.
  - Products (-W_d * img, bf16): column-split between DVE tensor_tensor
    (2x bf16 mode) and GPSIMD scalar_tensor_tensor ((w+0)*img, which the
    cost model rates at the higher default efficiency).
  - The 9-tap shifted accumulation runs on the otherwise-idle TensorEngine:
    identity-weighted matmuls accumulate shifted product slices into PSUM
    (bf16 moving data, 1 col/cycle). Each (view, chunk, channel) output is
    one 512-f32 PSUM bank; counts accumulate the weight tiles directly.
  - Normalize: DVE reciprocal_approx_fast on the (negated, eps-included)
    counts straight from PSUM, then one tensor_tensor multiply per
    (view, chunk) producing bf16 output, DMA'd out.

Sharding: pure data parallel over h (the scatter is along w only) - core m
takes h rows [m*128, (m+1)*128) for all batches. No halo, no communication.
I/O is bf16 for image and outputs (rel-err budget 2e-2; measured ~4e-3).
"""

import numpy as np

import concourse.bacc as bacc
import concourse.bass as bass
import concourse.mybir as mybir
import concourse.tile as tile
from concourse.bass_utils import run_bass_kernel_spmd

B, C, H, W = 4, 3, 1024, 1024
NCORES = 8
HS = H // NCORES  # 128 h-rows per core
NTAP = 9
CHUNK = 512  # PSUM bank = 512 f32 columns
NCHUNK = W // CHUNK
F32 = mybir.dt.float32
BF16 = mybir.dt.bfloat16
I32 = mybir.dt.int32
EPS = 1e-12

_ADD = mybir.AluOpType.add
_SUB = mybir.AluOpType.subtract
_MULT = mybir.AluOpType.mult
_MIN = mybir.AluOpType.min
_ABSMAX = mybir.AluOpType.abs_max
_EQ = mybir.AluOpType.is_equal
_ABS = mybir.ActivationFunctionType.Abs

CFG = {
    "io": 2, "v": 2, "t": 2, "w": NTAP + 1, "p": 3, "o": 3,
    # Product column split: DVE takes [0, spl), GPSIMD takes [spl, W).
    "spl": 800,
}


def _bcast_c(ap):
    """View a [HS, N] slice as [HS, C, N] by inserting a step-0 dim."""
    a = ap.ap
    return bass.AP(tensor=ap.tensor, offset=ap.offset, ap=[list(a[0]), [0, C], list(a[1])])


def _bcast_free(ap, n):
    """View a [HS, 1] slice as [HS, n] via a step-0 free dim."""
    a = ap.ap
    return bass.AP(tensor=ap.tensor, offset=ap.offset, ap=[list(a[0]), [0, n]])


def build_program(cfg: dict | None = None) -> bass.Bass:
    cfg = {**CFG, **(cfg or {})}
    spl = cfg["spl"]
    nc = bacc.Bacc()
    image = nc.dram_tensor("image", [B, C, HS, W], BF16, kind="ExternalInput")
    depth = nc.dram_tensor("depth", [B, HS, W], F32, kind="ExternalInput")
    left = nc.dram_tensor("left", [B, C, HS, W], BF16, kind="ExternalOutput")
    right = nc.dram_tensor("right", [B, C, HS, W], BF16, kind="ExternalOutput")

    with tile.TileContext(nc) as tc:
        with (
            tc.tile_pool(name="consts", bufs=1) as c_pool,
            tc.tile_pool(name="io", bufs=cfg["io"]) as io_pool,
            tc.tile_pool(name="vv", bufs=cfg["v"]) as v_pool,
            tc.tile_pool(name="tt", bufs=cfg["t"]) as t_pool,
            tc.tile_pool(name="wts", bufs=cfg["w"]) as w_pool,
            tc.tile_pool(name="prod", bufs=cfg["p"]) as p_pool,
            tc.tile_pool(name="norm", bufs=2) as n_pool,
            tc.tile_pool(name="outs", bufs=cfg["o"]) as o_pool,
            tc.psum_pool(name="acc0", bufs=1) as acc0_pool,
            tc.psum_pool(name="acc1", bufs=1) as acc1_pool,
            tc.psum_pool(name="cnt", bufs=1) as cnt_pool,
        ):
            # ---- one-time constants ----
            iota_i = c_pool.tile([HS, W], I32, tag="iotai")
            nc.gpsimd.iota(iota_i[:], [[1, W]], channel_multiplier=0)
            iota = c_pool.tile([HS, W], F32, tag="iota")
            nc.vector.tensor_copy(iota[:], iota_i[:])
            rowi = c_pool.tile([HS, 1], I32, tag="rowi")
            nc.gpsimd.iota(rowi[:], [[1, 1]], channel_multiplier=1)
            rowf = c_pool.tile([HS, 1], F32, tag="rowf")
            nc.vector.tensor_copy(rowf[:], rowi[:])
            ident = c_pool.tile([HS, HS], BF16, tag="ident")
            nc.vector.tensor_tensor(
                ident[:], iota[:, 0:HS], _bcast_free(rowf[:], HS), _EQ
            )
            negd = c_pool.tile([HS, NTAP], F32, tag="negd")
            for d in range(NTAP):
                nc.vector.memset(negd[:, d : d + 1], -float(d))

            for b in range(B):
                dep = io_pool.tile([HS, W], F32, tag="dep")
                nc.sync.dma_start(out=dep[:], in_=depth[b])
                img = io_pool.tile([HS, C, W], BF16, tag="img")
                nc.sync.dma_start(out=img[:], in_=image[b].transpose([1, 0, 2]))

                # Exact per-view fractional offsets (f32; matches reference
                # rounding of x +- depth bit-for-bit).
                vcat = v_pool.tile([HS, 2, W], F32, tag="vcat")
                s = t_pool.tile([HS, 2, W], F32, tag="t")
                nc.gpsimd.tensor_tensor(s[:, 0, :], dep[:], iota[:], _ADD)
                nc.gpsimd.tensor_tensor(vcat[:, 0, :], s[:, 0, :], iota[:], _SUB)
                s2 = t_pool.tile([HS, 2, W], F32, tag="t")
                nc.vector.tensor_tensor(s2[:, 0, :], iota[:], dep[:], _SUB)
                nc.vector.tensor_tensor(vcat[:, 1, :], iota[:], s2[:, 0, :], _SUB)

                # Negated hat weights for all taps, both views at once:
                # negw_d = min(|v - d| - 1, 0)   (tap 0: min(v - 1, -eps),
                # which injects the count epsilon for free).
                negw = []
                for d in range(NTAP):
                    wd = w_pool.tile([HS, 2, W], BF16, tag="w")
                    if d == 0:
                        nc.vector.tensor_scalar(wd[:], vcat[:], 1.0, 0.0, _SUB, _MIN)
                    else:
                        td = t_pool.tile([HS, 2, W], F32, tag="t")
                        nc.scalar.activation(
                            td[:], vcat[:], _ABS, bias=negd[:, d : d + 1], scale=1.0
                        )
                        nc.vector.tensor_scalar(wd[:], td[:], 1.0, 0.0, _SUB, _MIN)
                    negw.append(wd)
                # Tap-0 weight with -eps floor, used ONLY for the count
                # accumulation: injects exactly -eps into every count column
                # without touching acc (out = acc/max(count, eps) semantics).
                negw0e = w_pool.tile([HS, 2, W], BF16, tag="w0e")
                nc.vector.tensor_scalar(negw0e[:], vcat[:], 1.0, -EPS, _SUB, _MIN)

                outs = (left, right)
                for vi in range(2):
                    acc = [
                        pool.tile([HS, C, CHUNK], F32, name=f"acc{k}", tag=f"acc{k}")
                        for k, pool in enumerate((acc0_pool, acc1_pool))
                    ]
                    cnt = cnt_pool.tile([HS, NCHUNK, CHUNK], F32, tag="cnt")
                    for d in range(NTAP):
                        wv = negw[d][:, vi, :]
                        pd = p_pool.tile([HS, C, W], BF16, tag="p")
                        nc.vector.tensor_tensor(
                            pd[:, :, 0:spl], _bcast_c(wv[0:HS, 0:spl]), img[:, :, 0:spl], _MULT
                        )
                        nc.gpsimd.tensor_tensor(
                            pd[:, :, spl:W], _bcast_c(wv[0:HS, spl:W]),
                            img[:, :, spl:W], _MULT,
                        )
                        st, sp = (d == 0), (d == NTAP - 1)
                        for k in range(NCHUNK):
                            w0 = k * CHUNK
                            if vi == 0:  # left view: target j = src + d
                                lo = max(w0, d)
                                oa, ob = lo - w0, CHUNK
                                ra, rb = lo - d, w0 + CHUNK - d
                            else:  # right view: target j = src - d
                                hi = min(w0 + CHUNK, W - d)
                                oa, ob = 0, hi - w0
                                ra, rb = w0 + d, hi + d
                            for c in range(C):
                                nc.tensor.matmul(
                                    acc[k][:, c, oa:ob], ident[:], pd[:, c, ra:rb],
                                    start=st, stop=sp,
                                )
                            cw = negw0e if d == 0 else negw[d]
                            nc.tensor.matmul(
                                cnt[:, k, oa:ob], ident[:], cw[:, vi, :][0:HS, ra:rb],
                                start=st, stop=sp,
                            )

                    # Normalize: out = (-acc) * (-1/(count+eps)); counts in
                    # PSUM already hold -(count+eps).
                    rc = n_pool.tile([HS, NCHUNK, CHUNK], F32, tag="rc")
                    nc.vector.reciprocal_approx_fast(out=rc[:], in_=cnt[:])
                    for k in range(NCHUNK):
                        osb = o_pool.tile([HS, C, CHUNK], BF16, tag="osb")
                        nc.vector.tensor_tensor(
                            osb[:], acc[k][:], _bcast_c(rc[0:HS, k, :]), _MULT
                        )
                        nc.sync.dma_start(
                            out=outs[vi][b].transpose([1, 0, 2])[
                                :, :, k * CHUNK : (k + 1) * CHUNK
                            ],
                            in_=osb[:],
                        )
    nc.compile()
    return nc


_NC_CACHE = None


def _get_program():
    global _NC_CACHE
    if _NC_CACHE is None:
        _NC_CACHE = build_program()
    return _NC_CACHE


def kernel(image: np.ndarray, depth: np.ndarray):
    import ml_dtypes

    bf16 = np.dtype(ml_dtypes.bfloat16)
    image = np.ascontiguousarray(image, dtype=np.float32)
    depth = np.ascontiguousarray(depth, dtype=np.float32)
    assert image.shape == (B, C, H, W) and depth.shape == (B, H, W)
    image_b = image.astype(bf16)

    nc = _get_program()
    in_maps = []
    for m in range(NCORES):
        sl = slice(m * HS, (m + 1) * HS)
        in_maps.append(
            {
                "image": np.ascontiguousarray(image_b[:, :, sl, :]),
                "depth": np.ascontiguousarray(depth[:, sl, :]),
            }
        )
    # The axon-tunneled devices occasionally come up in a transient
    # unrecoverable/desynced state and recover on the next attempt - retry
    # once before giving up.
    try:
        res = run_bass_kernel_spmd(nc, in_maps, core_ids=list(range(NCORES)))
    except Exception:
        import time as _time

        _time.sleep(5.0)
        res = run_bass_kernel_spmd(nc, in_maps, core_ids=list(range(NCORES)))
    left = np.concatenate(
        [np.asarray(r["left"], dtype=np.float32) for r in res.results], axis=2
    )
    right = np.concatenate(
        [np.asarray(r["right"], dtype=np.float32) for r in res.results], axis=2
    )
    return left, right


# revision 68
# speedup vs baseline: 2.4263x; 1.2097x over previous
"""Dual-pixel depth-merge (forward splat) kernel for Trainium2, 8 NeuronCores.

Math: for integer pixel grid x, the reference computes pos = fl(x +- depth)
(f32-rounded), x0 = floor(pos), f = pos - x0. Define the per-view fractional
offsets
    v_l[i] = fl(i + depth[i]) - i   (exact f32 subtraction, in [0, 8])
    v_r[i] = i - fl(i - depth[i])   (exact f32 subtraction, in [0, 8])
Then each view's splat is a 9-tap shifted weighted sum with hat weights
    W_d = relu(1 - |v - d|), d = 0..8:
    count_l[j] = sum_d W_d[j-d]      acc_l[c,j] = sum_d (W_d*img_c)[j-d]
    count_r[j] = sum_d W_d[j+d]      acc_r[c,j] = sum_d (W_d*img_c)[j+d]
    out = acc / max(count, eps)

Implementation strategy (v2):
  - Work with NEGATED weights: -W_d = min(|v-d| - 1, 0). This needs no
    reverse-subtract and is exact in f32 (the zero set matches the reference
    bit-for-bit, which the count==0 -> divide-by-1 semantics require). Signs
    cancel at normalization: out = (-acc) * (-1/(count+eps)).
  - eps is folded into tap 0's weight: -W_0 = min(v-1, -eps) adds exactly
    -eps to every count column (and a negligible 1e-12 to acc).
  - Weights: ACT does t=|v-d| (abs, per-tap bias), DVE does the fused
    (t-1) min 0 tensor_scalar; both at [128, 2*1024] covering l+r at once.
  - Products (-W_d * img, bf16): column-split between DVE tensor_tensor
    (2x bf16 mode) and GPSIMD scalar_tensor_tensor ((w+0)*img, which the
    cost model rates at the higher default efficiency).
  - The 9-tap shifted accumulation runs on the otherwise-idle TensorEngine:
    identity-weighted matmuls accumulate shifted product slices into PSUM
    (bf16 moving data, 1 col/cycle). Each (view, chunk, channel) output is
    one 512-f32 PSUM bank; counts accumulate the weight tiles directly.
  - Normalize: DVE reciprocal_approx_fast on the (negated, eps-included)
    counts straight from PSUM, then one tensor_tensor multiply per
    (view, chunk) producing bf16 output, DMA'd out.

Sharding: pure data parallel over h (the scatter is along w only) - core m
takes h rows [m*128, (m+1)*128) for all batches. No halo, no communication.
I/O is bf16 for image and outputs (rel-err budget 2e-2; measured ~4e-3).
"""

import numpy as np

import concourse.bacc as bacc
import concourse.bass as bass
import concourse.mybir as mybir
import concourse.tile as tile
from concourse.bass_utils import run_bass_kernel_spmd

B, C, H, W = 4, 3, 1024, 1024
NCORES = 8
HS = H // NCORES  # 128 h-rows per core
NTAP = 9
CHUNK = 512  # PSUM bank = 512 f32 columns
NCHUNK = W // CHUNK
F32 = mybir.dt.float32
BF16 = mybir.dt.bfloat16
I32 = mybir.dt.int32
EPS = 1e-12

_ADD = mybir.AluOpType.add
_SUB = mybir.AluOpType.subtract
_MULT = mybir.AluOpType.mult
_DIV = mybir.AluOpType.divide
_MIN = mybir.AluOpType.min
_ABSMAX = mybir.AluOpType.abs_max
_EQ = mybir.AluOpType.is_equal
_ABS = mybir.ActivationFunctionType.Abs
_RELU = mybir.ActivationFunctionType.Relu

CFG = {
    "io": 2, "v": 2, "t": 2, "w": 2 * NTAP + 1, "p": 3, "o": 3,
    # Product column split: DVE takes [0, spl), GPSIMD takes [spl, W).
    # Batch 0 uses spl0 (its DVE also runs the weight relu steps).
    "spl": 800,
    "spl0": 720,
    "spl_last": 832,
    "pend_d": -1,
    "pend_split": 1,
}


def _bcast_c(ap):
    """View a [HS, N] slice as [HS, C, N] by inserting a step-0 dim."""
    a = ap.ap
    return bass.AP(tensor=ap.tensor, offset=ap.offset, ap=[list(a[0]), [0, C], list(a[1])])


def _bcast_free(ap, n):
    """View a [HS, 1] slice as [HS, n] via a step-0 free dim."""
    a = ap.ap
    return bass.AP(tensor=ap.tensor, offset=ap.offset, ap=[list(a[0]), [0, n]])


def build_program(cfg: dict | None = None) -> bass.Bass:
    cfg = {**CFG, **(cfg or {})}
    spl = cfg["spl"]
    nc = bacc.Bacc()
    image = nc.dram_tensor("image", [B, C, HS, W], BF16, kind="ExternalInput")
    depth = nc.dram_tensor("depth", [B, HS, W], F32, kind="ExternalInput")
    left = nc.dram_tensor("left", [B, C, HS, W], BF16, kind="ExternalOutput")
    right = nc.dram_tensor("right", [B, C, HS, W], BF16, kind="ExternalOutput")

    with tile.TileContext(nc) as tc:
        with (
            tc.tile_pool(name="consts", bufs=1) as c_pool,
            tc.tile_pool(name="io", bufs=cfg["io"]) as io_pool,
            tc.tile_pool(name="vv", bufs=cfg["v"]) as v_pool,
            tc.tile_pool(name="tt", bufs=cfg["t"]) as t_pool,
            tc.tile_pool(name="wts", bufs=cfg["w"]) as w_pool,
            tc.tile_pool(name="prod", bufs=cfg["p"]) as p_pool,
            tc.tile_pool(name="norm", bufs=2) as n_pool,
            tc.tile_pool(name="outs", bufs=cfg["o"]) as o_pool,
            tc.psum_pool(name="acc0", bufs=1) as acc0_pool,
            tc.psum_pool(name="acc1", bufs=1) as acc1_pool,
            tc.psum_pool(name="cnt", bufs=1) as cnt_pool,
        ):
            # ---- one-time constants ----
            iota_i = c_pool.tile([HS, W], I32, tag="iotai")
            nc.gpsimd.iota(iota_i[:], [[1, W]], channel_multiplier=0)
            iota = c_pool.tile([HS, W], F32, tag="iota")
            nc.vector.tensor_copy(iota[:], iota_i[:])
            rowi = c_pool.tile([HS, 1], I32, tag="rowi")
            nc.gpsimd.iota(rowi[:], [[1, 1]], channel_multiplier=1)
            rowf = c_pool.tile([HS, 1], F32, tag="rowf")
            nc.vector.tensor_copy(rowf[:], rowi[:])
            ident = c_pool.tile([HS, HS], BF16, tag="ident")
            nc.vector.tensor_tensor(
                ident[:], iota[:, 0:HS], _bcast_free(rowf[:], HS), _EQ
            )
            negd = c_pool.tile([HS, NTAP], F32, tag="negd")
            for d in range(NTAP):
                nc.vector.memset(negd[:, d : d + 1], -float(d))
            epsrow = c_pool.tile([HS, CHUNK], BF16, tag="epsrow")
            nc.vector.memset(epsrow[:], EPS)
            negepsrow = c_pool.tile([HS, CHUNK], BF16, tag="negepsrow")
            nc.vector.memset(negepsrow[:], -EPS)
            # Touch the ACT engine immediately so its activation-table load
            # (~2.7us) happens at t=0 instead of blocking the first weights.
            warm = c_pool.tile([HS, 1], F32, tag="warm")
            nc.scalar.activation(warm[:], negd[:, 0:1], _ABS, bias=negd[:, 0:1], scale=1.0)

            outs = (left, right)
            state: dict = {"imgs": {}, "negw": {}}

            def load_inputs(b):
                """DMA batch b's inputs and compute v (exact f32; matches
                reference rounding of x +- depth bit-for-bit)."""
                dep = io_pool.tile([HS, W], F32, tag="dep")
                nc.sync.dma_start(out=dep[:], in_=depth[b])
                img = io_pool.tile([HS, C, W], BF16, tag="img")
                nc.sync.dma_start(out=img[:], in_=image[b].transpose([1, 0, 2]))
                state["imgs"][b] = img

                vcat = v_pool.tile([HS, 2, W], F32, tag="vcat")
                s = t_pool.tile([HS, 2, W], F32, tag="t")
                nc.gpsimd.tensor_tensor(s[:, 0, :], dep[:], iota[:], _ADD)
                nc.gpsimd.tensor_tensor(vcat[:, 0, :], s[:, 0, :], iota[:], _SUB)
                s2 = t_pool.tile([HS, 2, W], F32, tag="t")
                nc.vector.tensor_tensor(s2[:, 0, :], iota[:], dep[:], _SUB)
                nc.vector.tensor_tensor(vcat[:, 1, :], iota[:], s2[:, 0, :], _SUB)
                state.setdefault("vcat", {})[b] = vcat
                state["negw"][b] = []
                dve_relu = b == 0
                state.setdefault("eps", {})[b] = negepsrow if dve_relu else epsrow

            def weights_part(b, taps):
                """Hat weights for a subset of taps, both views at once,
                bf16 out; the f32 zero set matches the reference
                bit-for-bit. Batch 0 uses NEGATED weights
                -W_d = min(|v-d|-1, 0) with the relu step on DVE (ACT can't
                pace the pipeline from a cold start); later batches compute
                +W_d = relu(1-|v-d|) fully on ACT. The sign cancels in
                acc/count at normalize; only the count-eps seed's sign must
                match."""
                vcat = state["vcat"][b]
                dve_relu = b == 0
                for d in taps:
                    wd = w_pool.tile([HS, 2, W], BF16, tag="w")
                    if d == 0:
                        if dve_relu:
                            nc.vector.tensor_scalar(wd[:], vcat[:], 1.0, 0.0, _SUB, _MIN)
                        else:
                            nc.scalar.activation(wd[:], vcat[:], _RELU, bias=1.0, scale=-1.0)
                    else:
                        td = t_pool.tile([HS, 2, W], F32, tag="t")
                        nc.scalar.activation(
                            td[:], vcat[:], _ABS, bias=negd[:, d : d + 1], scale=1.0
                        )
                        if dve_relu:
                            nc.vector.tensor_scalar(wd[:], td[:], 1.0, 0.0, _SUB, _MIN)
                        else:
                            nc.scalar.activation(wd[:], td[:], _RELU, bias=1.0, scale=-1.0)
                    state["negw"][b].append(wd)

            def load_and_weights(b):
                load_inputs(b)
                weights_part(b, range(NTAP))

            def norm_chunk(b, vi, acc, rc, k):
                osb = o_pool.tile([HS, C, CHUNK], BF16, tag="osb")
                nc.vector.tensor_tensor(
                    osb[:], acc[k][:], _bcast_c(rc[0:HS, k, :]), _MULT
                )
                nc.sync.dma_start(
                    out=outs[vi][b].transpose([1, 0, 2])[
                        :, :, k * CHUNK : (k + 1) * CHUNK
                    ],
                    in_=osb[:],
                )

            def normalize_parts(b, vi, acc, cnt):
                """Two-part normalize: part A = recip + chunk0 out; part B =
                chunk1 out. Splitting lets the next phase's first product
                slot between them on the DVE queue."""
                rcs = {}

                def part_a():
                    rc = n_pool.tile([HS, NCHUNK, CHUNK], F32, tag="rc")
                    nc.vector.reciprocal_approx_fast(out=rc[:], in_=cnt[:])
                    rcs["rc"] = rc
                    norm_chunk(b, vi, acc, rc, 0)

                def part_b():
                    norm_chunk(b, vi, acc, rcs["rc"], 1)

                return [part_a, part_b]

            load_and_weights(0)
            if B > 1:
                load_and_weights(1)
            pending = None  # deferred normalize of the previous phase
            for b in range(B):
                for vi in range(2):
                    img = state["imgs"][b]
                    negw = state["negw"][b]
                    acc = [
                        pool.tile([HS, C, CHUNK], F32, name=f"acc{k}", tag=f"acc{k}")
                        for k, pool in enumerate((acc0_pool, acc1_pool))
                    ]
                    cnt = cnt_pool.tile([HS, NCHUNK, CHUNK], F32, tag="cnt")

                    def emit_matmuls(d, pd, wv):
                        st, sp = (d == 0), (d == NTAP - 1)
                        for k in range(NCHUNK):
                            w0 = k * CHUNK
                            if vi == 0:  # left view: target j = src + d
                                lo = max(w0, d)
                                oa, ob = lo - w0, CHUNK
                                ra, rb = lo - d, w0 + CHUNK - d
                            else:  # right view: target j = src - d
                                hi = min(w0 + CHUNK, W - d)
                                oa, ob = 0, hi - w0
                                ra, rb = w0 + d, hi + d
                            for c in range(C):
                                nc.tensor.matmul(
                                    acc[k][:, c, oa:ob], ident[:], pd[:, c, ra:rb],
                                    start=st, stop=sp,
                                )
                            nc.tensor.matmul(
                                cnt[:, k, oa:ob], ident[:], wv[0:HS, ra:rb],
                                start=st, stop=sp,
                            )

                    # Products run one tap ahead of their matmuls so PE and
                    # the next phase's product never wait on each other at
                    # phase boundaries.
                    held = None
                    sb = cfg["spl0"] if b == 0 else spl
                    if (b, vi) == (B - 1, 1):
                        # Last phase: DVE finishes its half first so the
                        # tail normalize isn't gated on the slower GPSIMD
                        # product stream.
                        sb = cfg["spl_last"]
                    if cfg["pend_d"] < 0 and pending is not None:
                        pending.pop(0)()
                        if not cfg["pend_split"]:
                            while pending:
                                pending.pop(0)()
                    for d in range(NTAP):
                        wv = negw[d][:, vi, :]
                        pd = p_pool.tile([HS, C, W], BF16, tag="p")
                        nc.vector.tensor_tensor(
                            pd[:, :, 0:sb], _bcast_c(wv[0:HS, 0:sb]), img[:, :, 0:sb], _MULT
                        )
                        nc.gpsimd.tensor_tensor(
                            pd[:, :, sb:W], _bcast_c(wv[0:HS, sb:W]),
                            img[:, :, sb:W], _MULT,
                        )
                        if held is not None:
                            emit_matmuls(*held)
                        held = (d, pd, wv)
                        if d == 3:
                            # Inject the eps row into each count group
                            # mid-stream (the group is open by then).
                            for k in range(NCHUNK):
                                nc.tensor.matmul(
                                    cnt[:, k, :], ident[:], state["eps"][b][:],
                                    start=False, stop=False,
                                )
                        if pending and d >= max(cfg["pend_d"], 0):
                            # Drain any remaining normalize parts of the
                            # previous phase, one per tap slot.
                            pending.pop(0)()
                        if d == 5 and vi == 0 and b + 2 < B:
                            # Keep the ACT weight stream two batches ahead so
                            # products never starve on weights; split in two
                            # bursts so the normalize's ACT copies interleave
                            # without blocking the weight stream.
                            load_inputs(b + 2)
                            weights_part(b + 2, range(0, 5))
                        if d == 5 and vi == 1 and b + 2 < B:
                            weights_part(b + 2, range(5, NTAP))
                    emit_matmuls(*held)
                    pending = normalize_parts(b, vi, acc, cnt)
            for p in pending:
                p()
    nc.compile()
    return nc


_NC_CACHE = None


def _get_program():
    global _NC_CACHE
    if _NC_CACHE is None:
        _NC_CACHE = build_program()
    return _NC_CACHE


def kernel(image: np.ndarray, depth: np.ndarray):
    import ml_dtypes

    bf16 = np.dtype(ml_dtypes.bfloat16)
    image = np.ascontiguousarray(image, dtype=np.float32)
    depth = np.ascontiguousarray(depth, dtype=np.float32)
    assert image.shape == (B, C, H, W) and depth.shape == (B, H, W)
    image_b = image.astype(bf16)

    nc = _get_program()
    in_maps = []
    for m in range(NCORES):
        sl = slice(m * HS, (m + 1) * HS)
        in_maps.append(
            {
                "image": np.ascontiguousarray(image_b[:, :, sl, :]),
                "depth": np.ascontiguousarray(depth[:, sl, :]),
            }
        )
    # The axon-tunneled devices occasionally come up in a transient
    # unrecoverable/desynced state and recover on the next attempt - retry
    # once before giving up.
    try:
        res = run_bass_kernel_spmd(nc, in_maps, core_ids=list(range(NCORES)))
    except Exception:
        import time as _time

        _time.sleep(5.0)
        res = run_bass_kernel_spmd(nc, in_maps, core_ids=list(range(NCORES)))
    left = np.concatenate(
        [np.asarray(r["left"], dtype=np.float32) for r in res.results], axis=2
    )
    right = np.concatenate(
        [np.asarray(r["right"], dtype=np.float32) for r in res.results], axis=2
    )
    return left, right
